# revision 1
# baseline (speedup 1.0000x reference)
"""Trainium2 Bass kernel for nn_ConnectLoss.

loss = sum(relu(|x[:,j] - x[:,j-1]| - 1) * mask[:,j]) over j in [1, L).

Pure data-parallel over 8 NeuronCores: rows sharded 8192/core. Per
core, megatiles of 8x128 rows ([128, 8, 512] SBUF tiles) stream in on
two HWDGE queues (x via sync, mask via scalar, each 2-way split for
pacing); per megatile:
  DVE  tensor_tensor              d = x[:,1:] - x[:,:-1]
  ACT  activation(Abs, in-place)  d = |d|
  ACT  activation(Relu, bias=-1, in-place)  d = relu(d - 1)
  DVE  scalar_tensor_tensor       (d*1)*m with accum_out -> acc[:,t]
The kernel is DMA-bound (~33.5 MB/core streams at ~385 GB/s); the DVE
(~40%) and ACT (~30%) loads fit underneath. Host sums the 8 x
[128, n_mega] partials in float64.
"""
import sys

sys.path.insert(0, "/opt/trn_rl_repo")
import numpy as np

N_CORES = 8
M_ROWS = 65536
LENGTH = 512
ROWS_PER_CORE = M_ROWS // N_CORES
P = 128
BLOCKS = 8  # 128-row blocks fused per megatile (one DMA / DVE op each)

_nc_cache = None


def _build_nc(rows=ROWS_PER_CORE, length=LENGTH, blocks=BLOCKS):
    import concourse.tile as tile
    import concourse.mybir as mybir
    from concourse import bacc

    total_blocks = rows // P
    assert rows == total_blocks * P
    # Uniform megatiles (tapering measured slower).
    tail = []
    if total_blocks >= blocks + sum(tail):
        mid = total_blocks - sum(tail)
        schedule = [blocks] * (mid // blocks)
        rem = mid - (mid // blocks) * blocks
        if rem:
            schedule.append(rem)
        schedule += tail
    else:
        schedule = [blocks] * (total_blocks // blocks)
    n_mega = len(schedule)

    nc = bacc.Bacc(None)
    f32 = mybir.dt.float32
    x = nc.declare_dram_parameter("x", [rows, length], f32, isOutput=False)
    msk = nc.declare_dram_parameter("mask", [rows, length], f32, isOutput=False)
    out = nc.declare_dram_parameter("out", [P, n_mega], f32, isOutput=True)

    L1 = length - 1
    with tile.TileContext(nc) as tc:
        with (
            tc.tile_pool(name="xin", bufs=4) as xpool,
            tc.tile_pool(name="min", bufs=4) as mpool,
            tc.tile_pool(name="work", bufs=2) as wpool,
            tc.tile_pool(name="junk", bufs=1) as jpool,
            tc.tile_pool(name="acc", bufs=1) as apool,
        ):
            neg1 = apool.tile([P, 1], f32, tag="neg1")
            nc.vector.memset(neg1[:], -1.0)
            acc = apool.tile([P, n_mega], f32, tag="acc")
            junk = jpool.tile([P, blocks, length], f32, tag="junk")
            r0 = 0
            for t, nb in enumerate(schedule):
                xs = x[r0 * P : (r0 + nb) * P, :].rearrange(
                    "(b p) m -> p b m", p=P
                )
                ms = msk[r0 * P : (r0 + nb) * P, :].rearrange(
                    "(b p) m -> p b m", p=P
                )
                r0 += nb
                xt = xpool.tile([P, nb, length], f32, tag="xt")
                mt = mpool.tile([P, nb, length], f32, tag="mt")
                if nb > 1:
                    h = nb // 2
                    nc.sync.dma_start(xt[:, 0:h], xs[:, 0:h])
                    nc.sync.dma_start(xt[:, h:nb], xs[:, h:nb])
                    nc.scalar.dma_start(mt[:, 0:h], ms[:, 0:h])
                    nc.scalar.dma_start(mt[:, h:nb], ms[:, h:nb])
                else:
                    nc.sync.dma_start(xt[:], xs[:])
                    nc.scalar.dma_start(mt[:], ms[:])
                d = wpool.tile([P, nb, length], f32, tag="d")
                nc.vector.tensor_tensor(
                    d[:, :, 1:length],
                    xt[:, :, 1:length],
                    xt[:, :, 0:L1],
                    mybir.AluOpType.subtract,
                )
                nc.scalar.activation(
                    d[:, :, 1:length],
                    d[:, :, 1:length],
                    mybir.ActivationFunctionType.Abs,
                )
                nc.scalar.activation(
                    d[:, :, 1:length],
                    d[:, :, 1:length],
                    mybir.ActivationFunctionType.Relu,
                    bias=neg1[:, 0:1],
                    scale=1.0,
                )
                nc.vector.scalar_tensor_tensor(
                    junk[:, 0:nb, 1:length],
                    d[:, :, 1:length],
                    1.0,
                    mt[:, :, 1:length],
                    op0=mybir.AluOpType.mult,
                    op1=mybir.AluOpType.mult,
                    accum_out=acc[:, t : t + 1],
                )
            nc.sync.dma_start(out[:], acc[:])
    nc.compile()
    return nc


def _get_nc():
    global _nc_cache
    if _nc_cache is None:
        _nc_cache = _build_nc()
    return _nc_cache


def _finish(outs) -> np.ndarray:
    o = np.stack(outs).astype(np.float64)
    return np.asarray(o.sum(), dtype=np.float32)


def run_spmd(x, mask, trace: bool = False):
    """Returns (loss ndarray, BassKernelResults)."""
    from concourse.bass_utils import run_bass_kernel_spmd

    x = np.ascontiguousarray(np.asarray(x, dtype=np.float32))
    mask = np.ascontiguousarray(np.asarray(mask, dtype=np.float32))
    assert x.shape == (M_ROWS, LENGTH) and mask.shape == (M_ROWS, LENGTH)

    in_maps = [
        {
            "x": x[i * ROWS_PER_CORE : (i + 1) * ROWS_PER_CORE],
            "mask": mask[i * ROWS_PER_CORE : (i + 1) * ROWS_PER_CORE],
        }
        for i in range(N_CORES)
    ]
    res = run_bass_kernel_spmd(
        _get_nc(), in_maps, list(range(N_CORES)), trace=trace
    )
    loss = _finish([r["out"] for r in res.results])
    return loss, res


def kernel(x, mask) -> np.ndarray:
    loss, _ = run_spmd(x, mask, trace=False)
    return loss



# revision 16
# speedup vs baseline: 1.4025x; 1.4025x over previous
"""Trainium2 Bass kernel for nn_ConnectLoss.

loss = sum(relu(|x[:,j] - x[:,j-1]| - 1) * mask[:,j]) over j in [1, L).

Pure data-parallel over 8 NeuronCores: rows sharded 8192/core. Inputs
are downcast to bf16 on host (verified rel err ~5e-5 vs the 2e-2
gate), halving HBM traffic; per-core DRAM layout is partition-major
(partition p owns 64 adjacent rows) so every DMA moves one contiguous
8KB run per partition (128 descriptors / MB instead of 512).

Per 8-row megatile [128, 8, 512]:
  DVE tensor_tensor         dd[:, :, 0:511] = x[:,1:] - x[:,:-1]
  ACT activation(Abs)       dd = |dd|            (in-place)
  DVE tensor_scalar         dd = max(dd - 1, 0)  (in-place, (add,max))
  DVE tensor_tensor_reduce  dd * mshift, accum_out -> acc[:, t]
mshift is the mask DMA'd with a one-element shift so passes 2-3 run
full-width and 4B-aligned (2x 16-bit DVE mode); dd col 511 is zeroed
once per buffer so t col 511 is 0 and the junk mshift tail column
contributes nothing. Host sums the per-core [128, 8] partials in f64.
"""
import sys

sys.path.insert(0, "/opt/trn_rl_repo")
import numpy as np
import ml_dtypes

N_CORES = 8
M_ROWS = 65536
LENGTH = 512
ROWS_PER_CORE = M_ROWS // N_CORES
P = 128
RPP = ROWS_PER_CORE // P  # rows per partition (64)
BLOCKS = 8  # rows-per-partition fused per megatile
N_MEGA = RPP // BLOCKS
MASK_PAD = 64  # flat mask is read at +1 element offset

_nc_cache = None


def _build_nc():
    import concourse.tile as tile
    import concourse.mybir as mybir
    from concourse import bacc

    nc = bacc.Bacc(None)
    f32 = mybir.dt.float32
    bf16 = mybir.dt.bfloat16
    n_elem = ROWS_PER_CORE * LENGTH
    x = nc.declare_dram_parameter("x", [n_elem], bf16, isOutput=False)
    msk = nc.declare_dram_parameter(
        "mask", [n_elem + MASK_PAD], bf16, isOutput=False
    )
    out = nc.declare_dram_parameter("out", [P, N_MEGA], f32, isOutput=True)

    xv = x[0:n_elem].rearrange("(p r c) -> p r c", p=P, r=RPP, c=LENGTH)
    mv = msk[1 : 1 + n_elem].rearrange("(p r c) -> p r c", p=P, r=RPP, c=LENGTH)

    L1 = LENGTH - 1
    with tile.TileContext(nc) as tc:
        with (
            tc.tile_pool(name="xin", bufs=4) as xpool,
            tc.tile_pool(name="min", bufs=4) as mpool,
            tc.tile_pool(name="work", bufs=1) as wpool,
            tc.tile_pool(name="junk", bufs=1) as jpool,
            tc.tile_pool(name="acc", bufs=1) as apool,
        ):
            acc = apool.tile([P, N_MEGA], f32, tag="acc")
            junk = jpool.tile([P, BLOCKS, LENGTH], bf16, tag="junk")
            dds = [
                wpool.tile(
                    [P, BLOCKS, LENGTH], bf16, tag=f"dd{i}", name=f"dd{i}"
                )
                for i in range(3)
            ]
            for dd in dds:
                nc.vector.memset(dd[:, :, L1:LENGTH], 0.0)
            for t in range(N_MEGA):
                r0, r1 = t * BLOCKS, (t + 1) * BLOCKS
                xt = xpool.tile([P, BLOCKS, LENGTH], bf16, tag="xt")
                mt = mpool.tile([P, BLOCKS, LENGTH], bf16, tag="mt")
                nc.sync.dma_start(xt[:], xv[:, r0:r1, :])
                nc.scalar.dma_start(mt[:], mv[:, r0:r1, :])
                dd = dds[t % 3]
                nc.vector.tensor_tensor(
                    dd[:, :, 0:L1],
                    xt[:, :, 1:LENGTH],
                    xt[:, :, 0:L1],
                    mybir.AluOpType.subtract,
                )
                nc.scalar.activation(
                    dd[:],
                    dd[:],
                    mybir.ActivationFunctionType.Abs,
                )
                nc.vector.tensor_scalar(
                    dd[:],
                    dd[:],
                    -1.0,
                    0.0,
                    op0=mybir.AluOpType.add,
                    op1=mybir.AluOpType.max,
                )
                nc.vector.scalar_tensor_tensor(
                    junk[:],
                    dd[:],
                    1.0,
                    mt[:],
                    op0=mybir.AluOpType.mult,
                    op1=mybir.AluOpType.mult,
                    accum_out=acc[:, t : t + 1],
                )
            nc.sync.dma_start(out[:], acc[:])
    nc.compile()
    return nc


def _get_nc():
    global _nc_cache
    if _nc_cache is None:
        _nc_cache = _build_nc()
    return _nc_cache


def _finish(outs) -> np.ndarray:
    o = np.stack(outs).astype(np.float64)  # [cores, 128, N_MEGA]
    return np.asarray(o.sum(), dtype=np.float32)


def run_spmd(x, mask, trace: bool = False):
    """Returns (loss ndarray, BassKernelResults)."""
    from concourse.bass_utils import run_bass_kernel_spmd

    bf16 = ml_dtypes.bfloat16
    x = np.asarray(x, dtype=np.float32).astype(bf16)
    mask = np.asarray(mask, dtype=np.float32).astype(bf16)
    assert x.shape == (M_ROWS, LENGTH) and mask.shape == (M_ROWS, LENGTH)

    pad = np.zeros([MASK_PAD], dtype=bf16)
    in_maps = []
    for i in range(N_CORES):
        r0, r1 = i * ROWS_PER_CORE, (i + 1) * ROWS_PER_CORE
        in_maps.append(
            {
                "x": np.ascontiguousarray(x[r0:r1]).reshape(-1),
                "mask": np.concatenate(
                    [np.ascontiguousarray(mask[r0:r1]).reshape(-1), pad]
                ),
            }
        )
    res = run_bass_kernel_spmd(
        _get_nc(), in_maps, list(range(N_CORES)), trace=trace
    )
    loss = _finish([r["out"] for r in res.results])
    return loss, res


def kernel(x, mask) -> np.ndarray:
    loss, _ = run_spmd(x, mask, trace=False)
    return loss


# revision 23
# speedup vs baseline: 1.6030x; 1.1430x over previous
"""Trainium2 Bass kernel for nn_ConnectLoss.

loss = sum(relu(|x[:,j] - x[:,j-1]| - 1) * mask[:,j]) over j in [1, L).

Pure data-parallel over 8 NeuronCores: rows sharded 8192/core. Inputs
are downcast to bf16 on host (verified rel err ~5e-5 vs the 2e-2
gate), halving HBM traffic; per-core DRAM layout is partition-major
(partition p owns 64 adjacent rows) so every DMA moves one contiguous
8KB run per partition (128 descriptors / MB instead of 512).

Per 8-row megatile [128, 8, 512]:
  DVE tensor_tensor      dd[:, :, 0:511] = x[:,1:] - x[:,:-1]   (2x)
  ACT activation(Abs)    dd = |dd|            (in-place)
  DVE tensor_scalar      dd = max(dd - 1, 0)  (in-place, (add,max), 4x)
  DVE tensor_tensor      prod = dd * mshift   (2x)
  PE  matmul x8          ones[128,1].T @ prod[:, r, :] accumulating
                         into one PSUM bank [1, 512] f32
The masked sum runs on the otherwise-idle PE (the 1x-rate DVE
scalar_tensor_tensor accumulate was the bottleneck); one final ACT
Copy reduces PSUM -> acc[1, 1]. mshift is the mask DMA'd with a
one-element shift so DVE passes run full-width and 4B-aligned (16-bit
packed modes); dd col 511 is zeroed once per buffer so prod col 511
is 0 and the junk mshift tail column contributes nothing. Host sums
the 8 per-core scalars in f64.
"""
import sys

sys.path.insert(0, "/opt/trn_rl_repo")
import numpy as np
import ml_dtypes

N_CORES = 8
M_ROWS = 65536
LENGTH = 512
ROWS_PER_CORE = M_ROWS // N_CORES
P = 128
RPP = ROWS_PER_CORE // P  # rows per partition (64)
BLOCKS = 8  # rows-per-partition fused per megatile
N_MEGA = RPP // BLOCKS
MASK_PAD = 64  # flat mask is read at +1 element offset

_nc_cache = None


def _build_nc():
    import concourse.tile as tile
    import concourse.mybir as mybir
    from concourse import bacc

    nc = bacc.Bacc(None)
    f32 = mybir.dt.float32
    bf16 = mybir.dt.bfloat16
    n_elem = ROWS_PER_CORE * LENGTH
    x = nc.declare_dram_parameter("x", [n_elem], bf16, isOutput=False)
    msk = nc.declare_dram_parameter(
        "mask", [n_elem + MASK_PAD], bf16, isOutput=False
    )
    out = nc.declare_dram_parameter("out", [1, 1], f32, isOutput=True)

    xv = x[0:n_elem].rearrange("(p r c) -> p r c", p=P, r=RPP, c=LENGTH)
    mv = msk[1 : 1 + n_elem].rearrange("(p r c) -> p r c", p=P, r=RPP, c=LENGTH)

    L1 = LENGTH - 1
    with tile.TileContext(nc) as tc:
        with (
            tc.tile_pool(name="xin", bufs=4) as xpool,
            tc.tile_pool(name="min", bufs=4) as mpool,
            tc.tile_pool(name="work", bufs=1) as wpool,
            tc.tile_pool(name="prodp", bufs=3) as ppool,
            tc.tile_pool(name="acc", bufs=1) as apool,
            tc.psum_pool(name="ps", bufs=1) as pspool,
        ):
            acc = apool.tile([1, 1], f32, tag="acc")
            ones = apool.tile([P, 1], bf16, tag="ones")
            nc.vector.memset(ones[:], 1.0)
            ps = pspool.tile([1, LENGTH], f32, tag="ps")
            dds = [
                wpool.tile(
                    [P, BLOCKS, LENGTH], bf16, tag=f"dd{i}", name=f"dd{i}"
                )
                for i in range(3)
            ]
            for dd in dds:
                nc.vector.memset(dd[:, :, L1:LENGTH], 0.0)
            for t in range(N_MEGA):
                r0, r1 = t * BLOCKS, (t + 1) * BLOCKS
                xt = xpool.tile([P, BLOCKS, LENGTH], bf16, tag="xt")
                mt = mpool.tile([P, BLOCKS, LENGTH], bf16, tag="mt")
                nc.sync.dma_start(xt[:], xv[:, r0:r1, :])
                nc.scalar.dma_start(mt[:], mv[:, r0:r1, :])
                dd = dds[t % 3]
                nc.vector.tensor_tensor(
                    dd[:, :, 0:L1],
                    xt[:, :, 1:LENGTH],
                    xt[:, :, 0:L1],
                    mybir.AluOpType.subtract,
                )
                nc.scalar.activation(
                    dd[:],
                    dd[:],
                    mybir.ActivationFunctionType.Abs,
                )
                nc.vector.tensor_scalar(
                    dd[:],
                    dd[:],
                    -1.0,
                    0.0,
                    op0=mybir.AluOpType.add,
                    op1=mybir.AluOpType.max,
                )
                prod = ppool.tile([P, BLOCKS, LENGTH], bf16, tag="prod")
                nc.vector.tensor_tensor(
                    prod[:],
                    dd[:],
                    mt[:],
                    mybir.AluOpType.mult,
                )
                for r in range(BLOCKS):
                    nc.tensor.matmul(
                        ps[:],
                        ones[:],
                        prod[:, r, :],
                        start=(t == 0 and r == 0),
                        stop=(t == N_MEGA - 1 and r == BLOCKS - 1),
                    )
            fin = apool.tile([1, LENGTH], f32, tag="fin")
            nc.scalar.activation(
                fin[:],
                ps[:],
                mybir.ActivationFunctionType.Copy,
                accum_out=acc[:],
            )
            nc.sync.dma_start(out[:], acc[:])
    nc.compile()
    return nc


def _get_nc():
    global _nc_cache
    if _nc_cache is None:
        _nc_cache = _build_nc()
    return _nc_cache


def _finish(outs) -> np.ndarray:
    o = np.stack(outs).astype(np.float64)  # [cores, 1, 1]
    return np.asarray(o.sum(), dtype=np.float32)


def run_spmd(x, mask, trace: bool = False):
    """Returns (loss ndarray, BassKernelResults)."""
    from concourse.bass_utils import run_bass_kernel_spmd

    bf16 = ml_dtypes.bfloat16
    x = np.asarray(x, dtype=np.float32).astype(bf16)
    mask = np.asarray(mask, dtype=np.float32).astype(bf16)
    assert x.shape == (M_ROWS, LENGTH) and mask.shape == (M_ROWS, LENGTH)

    pad = np.zeros([MASK_PAD], dtype=bf16)
    in_maps = []
    for i in range(N_CORES):
        r0, r1 = i * ROWS_PER_CORE, (i + 1) * ROWS_PER_CORE
        in_maps.append(
            {
                "x": np.ascontiguousarray(x[r0:r1]).reshape(-1),
                "mask": np.concatenate(
                    [np.ascontiguousarray(mask[r0:r1]).reshape(-1), pad]
                ),
            }
        )
    res = run_bass_kernel_spmd(
        _get_nc(), in_maps, list(range(N_CORES)), trace=trace
    )
    loss = _finish([r["out"] for r in res.results])
    return loss, res


def kernel(x, mask) -> np.ndarray:
    loss, _ = run_spmd(x, mask, trace=False)
    return loss
